# revision 29
# baseline (speedup 1.0000x reference)
"""MultiHeadDiffAttention Trainium2 kernel (8 NeuronCores).

Sharding: core c = (b, g) with b = c // 4 (batch), g = c % 4 (head group).
Each core handles batch b and head-group g = 4 heads = 8 paired attention
streams (columns g*512:(g+1)*512 of the QKV projections).  The output
projection is column-sharded after an AllGather of the (transposed,
rms-scaled) attention outputs and an AllReduce of the per-row sum-of-squares.
The core emits out^T [S, T]; the host transposes while assembling.

Schedule (single deep pipeline, all matmuls bf16 with f32 PSUM):
  - x casts to bf16 DRAM scratch run row-block granular so DMA transposes
    and projection matmuls chase them with ~18us of lead-in.
  - v-projection, then k-projection (tch-major), then per t-chunk:
    q-projection for that chunk only, followed by the 4 couples' attention
    (logits -> exp on ACT -> AV with packed softmax-denominator column).
    logits of couple c+1 are issued before AV of couple c so the scalar
    engine never starves.
  - biases are folded into the PSUM->SBUF drains (no bias matmuls).
  - attention chunk is transposed SBUF->SBUF via the DMA xbar, then
    AllGather (batch group) + AllReduce of sumsq, chunked per t-chunk so
    collectives overlap the next chunk's compute.
  - output projection for chunk t is issued mid-attention of chunk t+1;
    rsqrt for the RMSNorm uses Exp(-0.5*Ln(.)) so the ACT engine stays on
    the one Exp/Ln table (no table thrash).
"""

import math
import os
import sys

if "/opt/trn_rl_repo" not in sys.path:
    sys.path.insert(0, "/opt/trn_rl_repo")

import numpy as np

B, T, D = 2, 2048, 2048
H, HD = 16, 64
LAMBDA_INIT = 0.8 - 0.6 * math.exp(-0.3 * 2)
RMS_EPS = 1e-6
N_CORES = 8
S = 512           # local stream columns (8 streams x 64)
NCP = 4           # local couples (heads)
KD = D // 128     # 16 contraction chunks
NTT = T // 128    # 16 T' tiles
NTCH = T // 512   # 4 t chunks

_RUNNER = None


def _build_nc():
    import concourse.bass as bass
    import concourse.tile as tile
    from concourse import bacc, mybir
    from contextlib import ExitStack

    BF = mybir.dt.bfloat16
    F32 = mybir.dt.float32
    AF = mybir.ActivationFunctionType
    OP = mybir.AluOpType

    nocoll = bool(os.environ.get("KERNEL_NOCOLL"))

    nc = bacc.Bacc("TRN2", num_devices=N_CORES)

    xq = nc.dram_tensor("xq", [T, D], BF, kind="ExternalInput")
    xk = nc.dram_tensor("xk", [T, D], BF, kind="ExternalInput")
    xv = nc.dram_tensor("xv", [T, D], BF, kind="ExternalInput")
    wq = nc.dram_tensor("wq", [D, S], BF, kind="ExternalInput")
    wk = nc.dram_tensor("wk", [D, S], BF, kind="ExternalInput")
    wv = nc.dram_tensor("wv", [D, S], BF, kind="ExternalInput")
    bq = nc.dram_tensor("bq", [S], F32, kind="ExternalInput")
    bk = nc.dram_tensor("bk", [S], F32, kind="ExternalInput")
    bv = nc.dram_tensor("bv", [S], F32, kind="ExternalInput")
    lq1 = nc.dram_tensor("lq1", [HD], F32, kind="ExternalInput")
    lk1 = nc.dram_tensor("lk1", [HD], F32, kind="ExternalInput")
    lq2 = nc.dram_tensor("lq2", [HD], F32, kind="ExternalInput")
    lk2 = nc.dram_tensor("lk2", [HD], F32, kind="ExternalInput")
    rmss = nc.dram_tensor("rmss", [S], F32, kind="ExternalInput")
    wo = nc.dram_tensor("wo", [D, S], BF, kind="ExternalInput")
    wob = nc.dram_tensor("wob", [S], F32, kind="ExternalInput")
    out = nc.dram_tensor("out", [S, T], F32, kind="ExternalOutput")

    repl = [[0, 1, 2, 3], [4, 5, 6, 7]]

    with tile.TileContext(nc) as tc:
        stack = ExitStack()
        singles = stack.enter_context(tc.tile_pool(name="singles", bufs=1))
        dram = stack.enter_context(tc.tile_pool(name="dram", bufs=1, space="DRAM"))

        # ---------- constants / scalars ----------
        # lambda = exp(sum(lq1*lk1)) - exp(sum(lq2*lk2)) + LAMBDA_INIT
        lvec = singles.tile([1, 4 * HD], F32)
        nc.gpsimd.dma_start(out=lvec[:, 0:HD], in_=lq1[:])
        nc.gpsimd.dma_start(out=lvec[:, HD:2 * HD], in_=lk1[:])
        nc.gpsimd.dma_start(out=lvec[:, 2 * HD:3 * HD], in_=lq2[:])
        nc.gpsimd.dma_start(out=lvec[:, 3 * HD:4 * HD], in_=lk2[:])
        lprod = singles.tile([1, 2, HD], F32)
        nc.vector.tensor_mul(lprod[:, 0, :], lvec[:, 0:HD], lvec[:, HD:2 * HD])
        nc.vector.tensor_mul(lprod[:, 1, :], lvec[:, 2 * HD:3 * HD],
                             lvec[:, 3 * HD:4 * HD])
        lsum = singles.tile([1, 2], F32)
        nc.vector.tensor_reduce(out=lsum, in_=lprod, axis=mybir.AxisListType.X,
                                op=OP.add)
        lexp = singles.tile([1, 2], F32)
        nc.scalar.activation(out=lexp, in_=lsum, func=AF.Exp)
        neg_lam1 = singles.tile([1, 1], F32)  # -(e1 - e2 + LAMBDA_INIT)
        nc.vector.tensor_sub(neg_lam1, lexp[:, 1:2], lexp[:, 0:1])
        nc.vector.tensor_scalar_add(neg_lam1, neg_lam1, -LAMBDA_INIT)
        neg_lam = singles.tile([128, 1], F32)
        nc.gpsimd.partition_broadcast(neg_lam, neg_lam1)

        # rms_scale / v-bias broadcast [128, 512] f32
        rms_b = singles.tile([128, S], F32)
        nc.gpsimd.dma_start(out=rms_b,
                            in_=bass.AP(tensor=rmss, offset=0,
                                        ap=[[0, 128], [1, S]]))
        bvb = singles.tile([128, S], F32)
        nc.gpsimd.dma_start(out=bvb,
                            in_=bass.AP(tensor=bv, offset=0,
                                        ap=[[0, 128], [1, S]]))
        # q/k biases as per-partition scalars [128, slot, couple]
        bqk = singles.tile([128, 2, NCP], F32)
        nc.gpsimd.dma_start(out=bqk[:, 0, :],
                          in_=bq[:].rearrange("(m p) -> p m", p=128))
        nc.gpsimd.dma_start(out=bqk[:, 1, :],
                          in_=bk[:].rearrange("(m p) -> p m", p=128))
        wob_sb = singles.tile([128, NCP], F32)
        nc.gpsimd.dma_start(out=wob_sb,
                          in_=wob[:].rearrange("(i p) -> p i", p=128))
        eps_sb = singles.tile([1, 1], F32)
        nc.vector.memset(eps_sb, RMS_EPS)
        lnl_sb = singles.tile([1, 1], F32)
        nc.vector.memset(lnl_sb, math.log(1.0 - LAMBDA_INIT))

        # ---------- DRAM scratch ----------
        ag_in = [dram.tile([S, 512], BF, name=f"ag_in{_t}") for _t in range(NTCH)]
        ag_out = [dram.tile([4 * S, 512], BF, name=f"ag_out{_t}")
                  for _t in range(NTCH)]
        ag_in3h = [dram.tile([S, 256], BF, name=f"ag_in3h{_h}") for _h in range(2)]
        ag_out3h = [dram.tile([4 * S, 256], BF, name=f"ag_out3h{_h}")
                    for _h in range(2)]
        ar_in = [dram.tile([512], F32, name=f"ar_in{_t}") for _t in range(NTCH)]
        ar_out = [dram.tile([512], F32, name=f"ar_out{_t}") for _t in range(NTCH)]

        # ---------- persistent SBUF pools ----------
        qk_pool = stack.enter_context(tc.tile_pool(name="qkp", bufs=2 * NCP))
        vh_pool = stack.enter_context(tc.tile_pool(name="vhp", bufs=NTT))
        wq_pool = stack.enter_context(tc.tile_pool(name="wqp", bufs=1))
        wo_pool = stack.enter_context(tc.tile_pool(name="wop", bufs=1))

        qhT = [qk_pool.tile([128, T], BF, tag="qk", name=f"qhT{m}")
               for m in range(NCP)]
        khT = [qk_pool.tile([128, T], BF, tag="qk", name=f"khT{m}")
               for m in range(NCP)]
        vh = [vh_pool.tile([128, NCP, 129], BF, tag="vh", name=f"vh{t}")
              for t in range(NTT)]
        for t in range(NTT):
            for c in range(NCP):
                nc.vector.memset(vh[t][:, c, 128:129], 1.0)
        wq_sb = wq_pool.tile([128, KD, S], BF, tag="wq", name="wq_sb")
        wo_sb = wo_pool.tile([128, KD, S], BF, tag="wo", name="wo_sb")
        wqs = [wq_sb[:, kd, :] for kd in range(KD)]
        wos = [wo_sb[:, kd, :] for kd in range(KD)]

        def wload_all(dst, src_t):
            nc.sync.dma_start(
                out=dst, in_=src_t[:].rearrange("(kd p) s -> p kd s", p=128))

        # ---------- phase A: v/k casts, transposes, projections ----------
        with (
            tc.tile_pool(name="xTvk", bufs=40) as xTvk_pool,
            tc.tile_pool(name="wvk", bufs=2) as wvk_pool,
            tc.tile_pool(name="psGa", bufs=4, space="PSUM") as psGa,
        ):
            wv_sb = wvk_pool.tile([128, KD, S], BF, tag="wvk", name="wv_sb")
            wk_sb = wvk_pool.tile([128, KD, S], BF, tag="wvk", name="wk_sb")
            wvs = [wv_sb[:, kd, :] for kd in range(KD)]
            wks = [wk_sb[:, kd, :] for kd in range(KD)]

            # inputs arrive bf16: transposes read them directly (no casts);
            # wv in two halves so the first transposes queue behind less data
            for hh in range(2):
                nc.sync.dma_start(
                    out=wv_sb[:, hh * 8:(hh + 1) * 8, :],
                    in_=wv[hh * 1024:(hh + 1) * 1024, :]
                    .rearrange("(kd p) s -> p kd s", p=128))

            def half_transposes(xsrc, tagname, h):
                tiles = []
                for kd in range(KD):
                    t_ = xTvk_pool.tile([128, 1024], BF, tag="xTvk",
                                        name=f"{tagname}_{h}_{kd}")
                    nc.sync.dma_start(
                        out=t_,
                        in_=xsrc[h * 1024:(h + 1) * 1024,
                                 kd * 128:(kd + 1) * 128],
                        transpose=True)
                    tiles.append(t_)
                return tiles

            # v projection: unit t needs half h = t//8
            xTv = [None, None]
            xTv[0] = half_transposes(xv, "xTv", 0)
            wload_all(wk_sb, wk)
            xTv[1] = half_transposes(xv, "xTv", 1)
            for t in range(NTT):
                h, j = t // 8, t % 8
                ps = psGa.tile([128, 512], F32, tag="psGa", name="psGv")
                for kd in range(KD):
                    nc.tensor.matmul(
                        ps, lhsT=xTv[h][kd][:, j * 128:(j + 1) * 128],
                        rhs=wvs[kd], start=(kd == 0), stop=(kd == KD - 1))
                nc.vector.tensor_add(
                    vh[t][:, :, 0:128],
                    ps[:].rearrange("p (c f) -> p c f", c=NCP),
                    bvb[:].rearrange("p (c f) -> p c f", c=NCP))

            # k projection (tch-major units)
            xTk = [None, None]
            xTk[0] = half_transposes(xk, "xTk", 0)
            wload_all(wq_sb, wq)
            xTk[1] = half_transposes(xk, "xTk", 1)
            for tch in range(NTCH):
                h, j = tch // 2, tch % 2
                for m in range(NCP):
                    ps = psGa.tile([128, 512], F32, tag="psGa", name="psGk")
                    for kd in range(KD):
                        nc.tensor.matmul(
                            ps, lhsT=wks[kd][:, m * 128:(m + 1) * 128],
                            rhs=xTk[h][kd][:, j * 512:(j + 1) * 512],
                            start=(kd == 0), stop=(kd == KD - 1))
                    nc.vector.tensor_scalar_add(
                        khT[m][:, tch * 512:(tch + 1) * 512], ps,
                        bqk[:, 1, m:m + 1])

        # ---------- attention + q-proj + collectives + outproj ----------
        with (
            tc.tile_pool(name="xTq", bufs=KD) as xTq_pool,
            tc.tile_pool(name="Eint", bufs=2) as Epool,
            tc.tile_pool(name="attn_sb", bufs=5) as attn_pool,
            tc.tile_pool(name="comb", bufs=4) as comb_pool,
            tc.tile_pool(name="ssq", bufs=6) as ssq_pool,
            tc.tile_pool(name="agT", bufs=6) as agT_pool,
            tc.tile_pool(name="aT", bufs=1) as aT_pool,
            tc.tile_pool(name="oT", bufs=2) as oT_pool,
            tc.tile_pool(name="rsb", bufs=2) as rsb_pool,
            tc.tile_pool(name="rs1", bufs=1) as rs1_pool,
            tc.tile_pool(name="psL", bufs=2, space="PSUM") as psL,
            tc.tile_pool(name="psA", bufs=2, space="PSUM") as psA,
            tc.tile_pool(name="psG", bufs=2, space="PSUM") as psG,
        ):
            xTq = [None] * KD

            def issue_qtransposes(tch):
                for kd in range(KD):
                    xTq[kd] = xTq_pool.tile([128, 512], BF, tag="xTq",
                                            name=f"xTq_{tch}_{kd}")
                    nc.sync.dma_start(
                        out=xTq[kd],
                        in_=xq[tch * 512:(tch + 1) * 512,
                               kd * 128:(kd + 1) * 128],
                        transpose=True)

            def issue_qproj_unit(tch, m):
                ps = psG.tile([128, 512], F32, tag="psG", name="psGq")
                for kd in range(KD):
                    nc.tensor.matmul(
                        ps, lhsT=wqs[kd][:, m * 128:(m + 1) * 128],
                        rhs=xTq[kd], start=(kd == 0), stop=(kd == KD - 1))
                nc.vector.tensor_scalar_add(
                    qhT[m][:, tch * 512:(tch + 1) * 512], ps,
                    bqk[:, 0, m:m + 1])

            def issue_qproj(tch):
                for m in range(NCP):
                    issue_qproj_unit(tch, m)

            def logits_pair(tch, c, E, tt):
                pl = psL.tile([128, 2, 512], F32, tag="psL", name="psL_t")
                nc.tensor.matmul(
                    pl[:, 0, :],
                    lhsT=khT[c][0:64, tt * 128:(tt + 1) * 128],
                    rhs=qhT[c][0:64, tch * 512:(tch + 1) * 512],
                    start=True, stop=True)
                nc.tensor.matmul(
                    pl[:, 1, :],
                    lhsT=khT[c][64:128, tt * 128:(tt + 1) * 128],
                    rhs=qhT[c][64:128, tch * 512:(tch + 1) * 512],
                    start=True, stop=True, tile_position=(64, 0))
                nc.scalar.activation(out=E[:, tt, :, :], in_=pl,
                                     func=AF.Exp, scale=0.125)

            def issue_logits(tch, c):
                E = Epool.tile([128, NTT, 2, 512], BF, tag="Eint", name="Eint_t")
                for tt in range(NTT):
                    logits_pair(tch, c, E, tt)
                return E

            def issue_av_ts(tch, c, E, ts, attn_sb, agT, ss_c):
                    pa = psA.tile([128, 2, 129], F32, tag="psA", name="psA_t")
                    for s_ in range(2):
                        for tt in range(NTT):
                            nc.tensor.matmul(
                                pa[:, s_, :],
                                lhsT=E[:, tt, s_, ts * 128:(ts + 1) * 128],
                                rhs=vh[tt][:, c, :],
                                start=(tt == 0), stop=(tt == NTT - 1))
                    r0 = comb_pool.tile([128, 1], F32, tag="r0", name="r0_t")
                    r1 = comb_pool.tile([128, 1], F32, tag="r1", name="r1_t")
                    nc.vector.reciprocal(r0, pa[:, 0, 128:129])
                    nc.vector.reciprocal(r1, pa[:, 1, 128:129])
                    nc.vector.tensor_mul(r1, r1, neg_lam)
                    t0 = comb_pool.tile([128, 128], F32, tag="t0", name="t0_t")
                    t1 = comb_pool.tile([128, 128], F32, tag="t1", name="t1_t")
                    nc.vector.tensor_scalar_mul(t0, pa[:, 0, 0:128], r0)
                    nc.vector.tensor_scalar_mul(t1, pa[:, 1, 0:128], r1)
                    nc.vector.tensor_add(t0, t0, t1)
                    nc.vector.tensor_mul(t1, t0, t0)  # reuse t1 as square
                    nc.vector.tensor_reduce(
                        out=ss_c[:, ts:ts + 1], in_=t1,
                        axis=mybir.AxisListType.X, op=OP.add)
                    nc.vector.tensor_mul(
                        attn_sb[ts][:, c * 128:(c + 1) * 128],
                        t0, rms_b[:, c * 128:(c + 1) * 128])
                    nc.sync.dma_start(
                        out=agT[c][:, ts * 128:(ts + 1) * 128],
                        in_=attn_sb[ts][:, c * 128:(c + 1) * 128],
                        transpose=True)

            def outproj_piece(tchx, aT, rsb, state, oc, j):
                # 4-matmul slice of the oc-th output-column unit
                if j == 0:
                    state[oc] = psG.tile([128, 512], F32, tag="psG",
                                         name="psGo")
                ps = state[oc]
                for kf in range(4 * j, 4 * j + 4):
                    nc.tensor.matmul(
                        ps, lhsT=wos[kf][:, oc * 128:(oc + 1) * 128],
                        rhs=aT[kf], start=(kf == 0), stop=(kf == KD - 1))
                if j == 3:
                    oT = oT_pool.tile([128, 512], F32, tag="oT", name="oT_t")
                    nc.vector.tensor_mul(oT, ps, rsb)
                    nc.vector.tensor_scalar_add(oT, oT, wob_sb[:, oc:oc + 1])
                    nc.sync.dma_start(
                        out=out[oc * 128:(oc + 1) * 128,
                                tchx * 512:(tchx + 1) * 512],
                        in_=oT)

            def issue_outproj(tchx, aT, rsb):
                state = {}
                for oc in range(NCP):
                    for j in range(4):
                        outproj_piece(tchx, aT, rsb, state, oc, j)

            def issue_outproj_head(tchx):
                # rsqrt path + gathered-activation loads for chunk tchx
                rs1 = rs1_pool.tile([1, 512], F32, tag="rs1", name="rs1_t")
                nc.gpsimd.dma_start(out=rs1, in_=ar_out[tchx][:])
                # (1-l0)/sqrt(ms+eps) = exp(-0.5*ln(ssq/D + eps) + ln(1-l0));
                # keeps ACT on the shared Exp/Ln table (Sqrt would thrash it)
                nc.scalar.activation(out=rs1, in_=rs1, func=AF.Ln,
                                     scale=1.0 / D, bias=eps_sb)
                nc.scalar.activation(out=rs1, in_=rs1, func=AF.Exp, scale=-0.5,
                                     bias=lnl_sb)
                rsb = rsb_pool.tile([128, 512], F32, tag="rsb", name="rsb_t")
                nc.gpsimd.partition_broadcast(rsb, rs1)
                aT_sb = aT_pool.tile([128, KD, 512], BF, tag="aT",
                                     name=f"aT_{tchx}")
                nc.gpsimd.dma_start(
                    out=aT_sb,
                    in_=ag_out[tchx][:].rearrange("(kf p) t -> p kf t", p=128))
                aT = [aT_sb[:, kf, :] for kf in range(KD)]
                return aT, rsb

            issue_qtransposes(0)

            out_head = None
            E_next = None
            for tch in range(NTCH):
                if tch == 0:
                    issue_qproj(0)
                    wload_all(wo_sb, wo)
                    E_next = issue_logits(0, 0)

                attn_sb = [attn_pool.tile([128, S], BF, tag="attn_sb",
                                          name=f"attn_sb_{tch}_{_t}")
                           for _t in range(4)]
                agT = [agT_pool.tile([128, 512], BF, tag="agT",
                                     name=f"agT_{tch}_{_f}")
                       for _f in range(4)]
                ss_c = [ssq_pool.tile([128, 4], F32, tag="ssc",
                                      name=f"ssc_{tch}_{_c}")
                        for _c in range(NCP)]

                for c in range(NCP):
                    E_cur = E_next
                    if c == 1 and tch < NTCH - 1:
                        issue_qtransposes(tch + 1)
                    op_state = {}
                    do_op = c == 2 and tch >= 1
                    if do_op:
                        out_head = issue_outproj_head(tch - 1)
                    if c < NCP - 1:
                        # interleave next couple's logits with this couple's
                        # AV and (at c==2) the previous chunk's outproj pieces
                        E_next = Epool.tile([128, NTT, 2, 512], BF, tag="Eint",
                                            name="Eint_t")
                        for tt in range(NTT):
                            logits_pair(tch, c + 1, E_next, tt)
                            if do_op:
                                outproj_piece(tch - 1, *out_head, op_state,
                                              tt // 4, tt % 4)
                            if tt % 4 == 3:
                                issue_av_ts(tch, c, E_cur, tt // 4,
                                            attn_sb, agT, ss_c[c])
                    elif tch < NTCH - 1:
                        # interleave next chunk's q-projection with final AV
                        for ts in range(4):
                            issue_qproj_unit(tch + 1, ts)
                            issue_av_ts(tch, c, E_cur, ts,
                                        attn_sb, agT, ss_c[c])
                    else:
                        for ts in range(4):
                            issue_av_ts(tch, c, E_cur, ts,
                                        attn_sb, agT, ss_c[c])
                if tch < NTCH - 1:
                    E_next = issue_logits(tch + 1, 0)

                # chunk complete: sumsq total + transposes + collectives
                nc.vector.tensor_add(ss_c[0], ss_c[0], ss_c[1])
                nc.vector.tensor_add(ss_c[2], ss_c[2], ss_c[3])
                nc.vector.tensor_add(ss_c[0], ss_c[0], ss_c[2])
                nc.gpsimd.dma_start(
                    out=ar_in[tch][:].rearrange("(t p) -> p t", p=128),
                    in_=ss_c[0])
                if tch == NTCH - 1:
                    for h in range(2):
                        for f in range(4):
                            nc.sync.dma_start(
                                out=ag_in3h[h][f * 128:(f + 1) * 128, :],
                                in_=agT[f][:, h * 256:(h + 1) * 256])
                else:
                    for f in range(4):
                        nc.sync.dma_start(
                            out=ag_in[tch][f * 128:(f + 1) * 128, :],
                            in_=agT[f])
                if tch == NTCH - 1:
                    if nocoll:
                        for h in range(2):
                            for _r in range(4):
                                nc.sync.dma_start(
                                    out=ag_out3h[h][_r * S:(_r + 1) * S, :],
                                    in_=ag_in3h[h][:, :])
                        nc.sync.dma_start(out=ar_out[tch][:],
                                          in_=ar_in[tch][:])
                    else:
                        for h in range(2):
                            nc.gpsimd.collective_compute(
                                "AllGather", OP.bypass, ins=[ag_in3h[h][:]],
                                outs=[ag_out3h[h][:]], replica_groups=repl)
                        nc.gpsimd.collective_compute(
                            "AllReduce", OP.add, ins=[ar_in[tch][:]],
                            outs=[ar_out[tch][:]], replica_groups=repl)
                elif nocoll:
                    # timing probe: replace collectives with local copies of
                    # equivalent local traffic (wrong data)
                    for _r in range(4):
                        nc.gpsimd.dma_start(
                            out=ag_out[tch][_r * S:(_r + 1) * S, :],
                            in_=ag_in[tch][:, :])
                    nc.gpsimd.dma_start(out=ar_out[tch][:], in_=ar_in[tch][:])
                else:
                    nc.gpsimd.collective_compute(
                        "AllGather", OP.bypass, ins=[ag_in[tch][:]],
                        outs=[ag_out[tch][:]], replica_groups=repl)
                    nc.gpsimd.collective_compute(
                        "AllReduce", OP.add, ins=[ar_in[tch][:]],
                        outs=[ar_out[tch][:]], replica_groups=repl)

            # final chunk: rs path + per-half gathered loads and outproj
            tl = NTCH - 1
            rs1 = rs1_pool.tile([1, 512], F32, tag="rs1", name="rs1_f")
            nc.gpsimd.dma_start(out=rs1, in_=ar_out[tl][:])
            nc.scalar.activation(out=rs1, in_=rs1, func=AF.Ln,
                                 scale=1.0 / D, bias=eps_sb)
            nc.scalar.activation(out=rs1, in_=rs1, func=AF.Exp, scale=-0.5,
                                 bias=lnl_sb)
            rsb = rsb_pool.tile([128, 512], F32, tag="rsb", name="rsb_f")
            nc.gpsimd.partition_broadcast(rsb, rs1)
            for h in range(2):
                aTh = aT_pool.tile([128, KD, 256], BF, tag="aTh",
                                   name=f"aT3_{h}")
                nc.gpsimd.dma_start(
                    out=aTh,
                    in_=ag_out3h[h][:].rearrange("(kf p) t -> p kf t", p=128))
                for oc in range(NCP):
                    ps = psG.tile([128, 256], F32, tag="psGh", name="psGoh")
                    for kf in range(KD):
                        nc.tensor.matmul(
                            ps, lhsT=wos[kf][:, oc * 128:(oc + 1) * 128],
                            rhs=aTh[:, kf, :], start=(kf == 0),
                            stop=(kf == KD - 1))
                    oT = oT_pool.tile([128, 256], F32, tag="oTh", name="oTh_t")
                    nc.vector.tensor_mul(
                        oT, ps, rsb[:, h * 256:(h + 1) * 256])
                    nc.vector.tensor_scalar_add(oT, oT, wob_sb[:, oc:oc + 1])
                    nc.sync.dma_start(
                        out=out[oc * 128:(oc + 1) * 128,
                                tl * 512 + h * 256:tl * 512 + (h + 1) * 256],
                        in_=oT)

        stack.close()

    nc.compile()
    return nc


class _Runner:
    """Caches the compiled NEFF + jitted shard_map callable."""

    def __init__(self):
        import jax
        import jax.numpy as jnp  # noqa: F401
        from jax.sharding import Mesh, PartitionSpec
        from jax.experimental.shard_map import shard_map
        from concourse import bass2jax, mybir

        self.jax = jax
        nc = _build_nc()
        self.nc = nc
        bass2jax.install_neuronx_cc_hook()

        partition_name = (nc.partition_id_tensor.name
                          if nc.partition_id_tensor else None)
        in_names, out_names, out_avals, zero_outs = [], [], [], []
        for alloc in nc.m.functions[0].allocations:
            if not isinstance(alloc, mybir.MemoryLocationSet):
                continue
            name = alloc.memorylocations[0].name
            if alloc.kind == "ExternalInput":
                if name != partition_name:
                    in_names.append(name)
            elif alloc.kind == "ExternalOutput":
                out_names.append(name)
                shape = tuple(alloc.tensor_shape)
                dtype = mybir.dt.np(alloc.dtype)
                out_avals.append(jax.core.ShapedArray(shape, dtype))
                zero_outs.append(np.zeros(shape, dtype))
        self.in_names = in_names
        self.out_names = out_names
        self.zero_outs = zero_outs
        n_params = len(in_names)
        n_outs = len(out_avals)
        full_in_names = list(in_names) + list(out_names)
        if partition_name is not None:
            full_in_names.append(partition_name)

        def _body(*args):
            operands = list(args)
            if partition_name is not None:
                operands.append(bass2jax.partition_id_tensor())
            outs = bass2jax._bass_exec_p.bind(
                *operands,
                out_avals=tuple(out_avals),
                in_names=tuple(full_in_names),
                out_names=tuple(out_names),
                lowering_input_output_aliases=(),
                sim_require_finite=True,
                sim_require_nnan=True,
                nc=nc,
            )
            return tuple(outs)

        devices = jax.devices()[:N_CORES]
        assert len(devices) == N_CORES
        mesh = Mesh(np.asarray(devices), ("core",))
        in_specs = (PartitionSpec("core"),) * (n_params + n_outs)
        out_specs = (PartitionSpec("core"),) * n_outs
        self.fn = jax.jit(
            shard_map(_body, mesh=mesh, in_specs=in_specs,
                      out_specs=out_specs, check_rep=False),
            keep_unused=True,
        )
        self.mesh = mesh

    def device_args(self, in_maps):
        """Concat per-core inputs on axis 0 and push to devices."""
        concat = [np.concatenate([np.asarray(in_maps[c][n])
                                  for c in range(N_CORES)], axis=0)
                  for n in self.in_names]
        concat += [np.zeros((N_CORES * z.shape[0], *z.shape[1:]), z.dtype)
                   for z in self.zero_outs]
        return concat

    def run(self, args):
        outs = self.fn(*args)
        res = [np.asarray(o) for o in outs]
        per_core = []
        for c in range(N_CORES):
            m = {}
            for i, name in enumerate(self.out_names):
                shp = self.zero_outs[i].shape
                m[name] = res[i].reshape(N_CORES, *shp)[c]
            per_core.append(m)
        return per_core


def _get_runner():
    global _RUNNER
    if _RUNNER is None:
        _RUNNER = _Runner()
    return _RUNNER


def _make_in_maps(inputs):
    import ml_dtypes
    bf16 = ml_dtypes.bfloat16
    q = np.asarray(inputs["query"], np.float32).astype(bf16)
    k = np.asarray(inputs["key_in"], np.float32).astype(bf16)
    v = np.asarray(inputs["value"], np.float32).astype(bf16)
    wq_k = np.asarray(inputs["wq_k"], np.float32).astype(bf16)
    wk_k = np.asarray(inputs["wk_k"], np.float32).astype(bf16)
    wv_k = np.asarray(inputs["wv_k"], np.float32).astype(bf16)
    wq_b = np.asarray(inputs["wq_b"], np.float32)
    wk_b = np.asarray(inputs["wk_b"], np.float32)
    wv_b = np.asarray(inputs["wv_b"], np.float32)
    rms = np.asarray(inputs["rms_scale"], np.float32)
    wo_k = np.asarray(inputs["wo_k"], np.float32).astype(ml_dtypes.bfloat16)
    wo_b = np.asarray(inputs["wo_b"], np.float32)
    in_maps = []
    for c in range(N_CORES):
        b, g = c // 4, c % 4
        sl = slice(g * S, (g + 1) * S)
        in_maps.append({
            "xq": np.ascontiguousarray(q[b]),
            "xk": np.ascontiguousarray(k[b]),
            "xv": np.ascontiguousarray(v[b]),
            "wq": np.ascontiguousarray(wq_k[:, sl]),
            "wk": np.ascontiguousarray(wk_k[:, sl]),
            "wv": np.ascontiguousarray(wv_k[:, sl]),
            "bq": np.ascontiguousarray(wq_b[sl]),
            "bk": np.ascontiguousarray(wk_b[sl]),
            "bv": np.ascontiguousarray(wv_b[sl]),
            "lq1": np.asarray(inputs["lq1"], np.float32),
            "lk1": np.asarray(inputs["lk1"], np.float32),
            "lq2": np.asarray(inputs["lq2"], np.float32),
            "lk2": np.asarray(inputs["lk2"], np.float32),
            "rmss": np.ascontiguousarray(rms[sl]),
            "wo": np.ascontiguousarray(wo_k[:, sl]),
            "wob": np.ascontiguousarray(wo_b[sl]),
        })
    return in_maps


def _assemble(per_core):
    full = np.zeros((B, T, D), np.float32)
    for c in range(N_CORES):
        b, g = c // 4, c % 4
        full[b, :, g * S:(g + 1) * S] = per_core[c]["out"].T
    return full


def kernel(**inputs):
    runner = _get_runner()
    args = runner.device_args(_make_in_maps(inputs))
    return _assemble(runner.run(args))
